# revision 1
# baseline (speedup 1.0000x reference)
"""Causal self-attention (B=4, T=2048, D=1024, H=16) on 8 trn2 NeuronCores.

Sharding: Megatron-style tensor parallel over heads (TP=2) x data parallel
over batch (DP=4). Core c handles batch c//2 and head-group c%2 (8 heads).
Each core computes its QKV projection slice, causal attention for its 8
heads, and a partial output projection; the host sums the two TP partials
per batch and adds b_proj.

All matmuls run in fp16 (fp32 PSUM accumulation); softmax runs in fp32 on
the scalar engine (exp) / DVE (reciprocal).
"""
import sys

sys.path.insert(0, "/opt/trn_rl_repo")

import numpy as np
import ml_dtypes

import concourse.bass as bass
import concourse.tile as tile
from concourse import bacc, mybir
from concourse.bass_utils import run_bass_kernel_spmd

B, T, D, H = 4, 2048, 1024, 16
HD = 64            # head dim
HL = 8             # heads per core (TP=2)
DL = HL * HD       # 512 local qkv width
KCH = D // 128     # 8 contraction chunks for QKV
TCH = T // 128     # 16 T chunks of 128
TB = T // 512      # 4 T blocks of 512
F16 = mybir.dt.float16
F32 = mybir.dt.float32
NEG = -1.0e30

_cache = {}


def _build():
    nc = bacc.Bacc("TRN2", target_bir_lowering=False, num_devices=8)

    xT = nc.dram_tensor("xT", [D, T], F16, kind="ExternalInput")
    wqk = nc.dram_tensor("wqk", [D, 2 * DL], F16, kind="ExternalInput")
    bqk = nc.dram_tensor("bqk", [128, 2 * DL // 128], F32, kind="ExternalInput")
    wv = nc.dram_tensor("wv", [D, DL], F16, kind="ExternalInput")
    bv = nc.dram_tensor("bv", [1, DL], F32, kind="ExternalInput")
    wp = nc.dram_tensor("wp", [DL, D], F16, kind="ExternalInput")
    tri = nc.dram_tensor("tri", [128, 128], F32, kind="ExternalInput")
    out = nc.dram_tensor("out", [T, D], F32, kind="ExternalOutput")

    with tile.TileContext(nc) as tc:
        with (
            tc.tile_pool(name="const", bufs=1) as const,
            tc.tile_pool(name="acts", bufs=1) as acts,
            tc.tile_pool(name="work", bufs=4) as work,
            tc.tile_pool(name="small", bufs=3) as small,
            tc.tile_pool(name="outp", bufs=3) as outp,
            tc.tile_pool(name="psb", bufs=3, space="PSUM") as psb,
            tc.tile_pool(name="psy", bufs=2, space="PSUM") as psy,
        ):
            # ---- resident inputs ----
            xT_sb = []
            wqk_sb = []
            wv_sb = []
            for k in range(KCH):
                xt = const.tile([128, T], F16, name=f"xT{k}", tag=f"xT{k}")
                nc.sync.dma_start(out=xt[:, 0:T // 2],
                                  in_=xT[128 * k:128 * (k + 1), 0:T // 2])
                xT_sb.append(xt)
                wq = const.tile([128, 2 * DL], F16, name=f"wqk{k}", tag=f"wqk{k}")
                nc.sync.dma_start(out=wq, in_=wqk[128 * k:128 * (k + 1), :])
                wqk_sb.append(wq)
                wvt = const.tile([128, DL], F16, name=f"wv{k}", tag=f"wv{k}")
                nc.gpsimd.dma_start(out=wvt, in_=wv[128 * k:128 * (k + 1), :])
                wv_sb.append(wvt)
            for k in range(KCH):
                nc.sync.dma_start(out=xT_sb[k][:, T // 2:T],
                                  in_=xT[128 * k:128 * (k + 1), T // 2:T])
            wp_sb = []
            for c in range(DL // 128):
                wpt = const.tile([128, D], F16, name=f"wp{c}", tag=f"wp{c}")
                nc.gpsimd.dma_start(out=wpt, in_=wp[128 * c:128 * (c + 1), :])
                wp_sb.append(wpt)
            bqk_sb = const.tile([128, 2 * DL // 128], F32)
            nc.gpsimd.dma_start(out=bqk_sb, in_=bqk[:, :])
            bv_sb = const.tile([1, DL], F32)
            nc.gpsimd.dma_start(out=bv_sb, in_=bv[:, :])
            tri_sb = const.tile([128, 128], F32)
            nc.gpsimd.dma_start(out=tri_sb, in_=tri[:, :])
            bvb_sb = const.tile([128, DL], F32)
            nc.gpsimd.partition_broadcast(bvb_sb, bv_sb)

            # ---- persistent activations ----
            qT_sb = [acts.tile([128, T], F16, name=f"qT{c}", tag=f"qT{c}")
                     for c in range(4)]
            # kT stored per head, zero-padded to K=128: head 2c occupies
            # partitions 0:64 (64:128 zero), head 2c+1 partitions 64:128
            # (0:64 zero).  This keeps every S matmul full-array (no
            # row-group masking, which stops the PE activity monitor from
            # registering "busy" and parks the clock at half rate).
            kT2_sb = [acts.tile([128, T], F16, name=f"kT2h{h}", tag=f"kT2h{h}")
                      for h in range(HL)]
            for h in range(HL):
                z0, z1 = (64, 128) if h % 2 == 0 else (0, 64)
                nc.gpsimd.memset(kT2_sb[h][z0:z1, :], 0.0)
            vaug = [acts.tile([128, HL * (HD + 1)], F16, name=f"va{t}",
                              tag=f"va{t}") for t in range(TCH)]
            yT_sb = [acts.tile([128, T], F16, name=f"yT{c}", tag=f"yT{c}")
                     for c in range(4)]

            # ---- streamed pipeline over T-block pairs (tb2 = 1024 rows) ----
            # Each tb2 round: QKV projection for the block, then causal
            # attention for q0 blocks 2*tb2 and 2*tb2+1, then the output
            # projection for those rows.  Later rounds' projection matmuls
            # (PE) overlap earlier rounds' softmax exps (scalar engine).
            for tb2 in range(TB // 2):
                # -- qT / kT = (w_slice)^T @ xT for this block --
                for cc in range(2 * DL // 128):
                    ps_w = psb.tile([128, 1024], F32, name="psB", tag="psB")
                    for half in range(2):
                        tb = 2 * tb2 + half
                        for k in range(KCH):
                            nc.tensor.matmul(
                                ps_w[:, 512 * half:512 * (half + 1)],
                                wqk_sb[k][:, 128 * cc:128 * (cc + 1)],
                                xT_sb[k][:, 512 * tb:512 * (tb + 1)],
                                start=(k == 0), stop=(k == KCH - 1),
                            )
                    tbs = slice(1024 * tb2, 1024 * (tb2 + 1))
                    if cc < 4:
                        nc.vector.tensor_scalar_add(
                            out=qT_sb[cc][:, tbs],
                            in0=ps_w,
                            scalar1=bqk_sb[:, cc:cc + 1],
                        )
                    else:
                        hA = 2 * (cc - 4)
                        nc.vector.tensor_scalar_add(
                            out=kT2_sb[hA][0:64, tbs],
                            in0=ps_w[0:64, :],
                            scalar1=bqk_sb[0:64, cc:cc + 1],
                        )
                        nc.vector.tensor_scalar_add(
                            out=kT2_sb[hA + 1][64:128, tbs],
                            in0=ps_w[64:128, :],
                            scalar1=bqk_sb[64:128, cc:cc + 1],
                        )
                # -- v (natural layout) + ones column for this block --
                for t2 in range(4 * tb2, 4 * (tb2 + 1)):
                    ps_w = psb.tile([128, 1024], F32, name="psB", tag="psB")
                    for half in range(2):
                        t = 2 * t2 + half
                        hs = slice(512 * half, 512 * (half + 1))
                        for k in range(KCH):
                            nc.tensor.matmul(
                                ps_w[:, hs],
                                xT_sb[k][:, 128 * t:128 * (t + 1)],
                                wv_sb[k],
                                start=(k == 0), stop=(k == KCH - 1),
                            )
                    for half in range(2):
                        t = 2 * t2 + half
                        va = vaug[t]
                        va3 = va.rearrange("p (h c) -> p h c", c=HD + 1)
                        nc.vector.tensor_add(
                            va3[:, :, 0:HD],
                            ps_w[:, 512 * half:512 * (half + 1)].rearrange(
                                "p (h d) -> p h d", d=HD),
                            bvb_sb.rearrange("p (h d) -> p h d", d=HD),
                        )
                        nc.gpsimd.memset(va3[:, :, HD], 1.0)
                # -- attention for q0 in {2*tb2, 2*tb2+1}, all head pairs --
                for q0 in (2 * tb2, 2 * tb2 + 1):
                    for c in range(4):
                        ntiles = 4 * q0 + 4
                        ps_ys = [psy.tile([HD + 1, 512], F32, name="psY",
                                          tag="psY") for p in range(2)]
                        for t in range(ntiles):
                            m = t - 4 * q0
                            lo = 128 * m if m > 0 else 0
                            # both heads' scores side by side, 2-bank psum
                            ps_s = psb.tile([128, 1024], F32, name="psB",
                                            tag="psB")
                            for p in range(2):  # the two heads 2c, 2c+1
                                # full-K matmul: zero-padded kT kills the
                                # other head's rows of qT
                                nc.tensor.matmul(
                                    ps_s[:, 512 * p + lo:512 * (p + 1)],
                                    kT2_sb[2 * c + p][:,
                                                      128 * t:128 * (t + 1)],
                                    qT_sb[c][:,
                                             512 * q0 + lo:512 * (q0 + 1)],
                                    start=True, stop=True,
                                )
                            if m >= 0:
                                # one masked add over both heads via 3D AP
                                seg = ps_s.rearrange("p (u f) -> p u f", u=2)
                                nc.vector.tensor_add(
                                    seg[:, :, lo:lo + 128],
                                    seg[:, :, lo:lo + 128],
                                    tri_sb.unsqueeze(1).broadcast_to(
                                        [128, 2, 128]),
                                )
                            es = work.tile([128, 1024], F16, name="es",
                                           tag="es")
                            nc.scalar.activation(
                                out=es[:, lo:1024],
                                in_=ps_s[:, lo:1024],
                                func=mybir.ActivationFunctionType.Exp,
                            )
                            for p in range(2):
                                h = 2 * c + p
                                nc.tensor.matmul(
                                    ps_ys[p][:, lo:512],
                                    vaug[t][:,
                                            (HD + 1) * h:(HD + 1) * (h + 1)],
                                    es[:, 512 * p + lo:512 * (p + 1)],
                                    start=(t == 0), stop=(t == ntiles - 1),
                                )
                        for p in range(2):
                            poff = 64 * p
                            # free the PV psum quickly: copy to SBUF, then
                            # normalize off SBUF.  partition_broadcast needs
                            # its source at base partition 0 (dn copy).
                            ys = small.tile([64, 512], F32, name="ys",
                                            tag="ys")
                            nc.vector.tensor_copy(ys, ps_ys[p][0:HD, :])
                            dn = small.tile([1, 512], F32, name="dn",
                                            tag="dn")
                            nc.vector.tensor_copy(dn, ps_ys[p][HD:HD + 1, :])
                            dnb = small.tile([64, 512], F32, name="dnb",
                                             tag="dnb")
                            nc.gpsimd.partition_broadcast(dnb, dn)
                            rcb = small.tile([64, 512], F32, name="rcb",
                                             tag="rcb")
                            nc.vector.reciprocal_approx_fast(rcb, dnb)
                            nc.vector.tensor_mul(
                                yT_sb[c][poff:poff + 64,
                                         512 * q0:512 * (q0 + 1)],
                                ys,
                                rcb,
                            )
                # -- partial out projection for this block's rows --
                for t in range(8 * tb2, 8 * (tb2 + 1)):
                    ps_o = psb.tile([128, 1024], F32, name="psB", tag="psB")
                    for nb in range(D // 512):
                        for c in range(DL // 128):
                            nc.tensor.matmul(
                                ps_o[:, 512 * nb:512 * (nb + 1)],
                                yT_sb[c][:, 128 * t:128 * (t + 1)],
                                wp_sb[c][:, 512 * nb:512 * (nb + 1)],
                                start=(c == 0), stop=(c == DL // 128 - 1),
                            )
                    ob = outp.tile([128, 1024], F32, name="ob", tag="ob")
                    nc.vector.tensor_copy(ob, ps_o)
                    nc.sync.dma_start(
                        out=out[128 * t:128 * (t + 1), :],
                        in_=ob,
                    )

    nc.finalize()
    return nc


def _enable_trace_hooks():
    """Inject antenv.axon_hooks + no-op artifact upload so that
    run_bass_kernel_spmd(trace=True) works under axon in this image."""
    import types
    import antenv

    if "antenv.axon_hooks" not in sys.modules:
        mod = types.ModuleType("antenv.axon_hooks")
        state = {"hook": None}
        mod.set_axon_ntff_profile_hook = lambda h: state.__setitem__("hook", h)
        mod.get_axon_ntff_profile_hook = lambda: state["hook"]
        sys.modules["antenv.axon_hooks"] = mod
        antenv.axon_hooks = mod
        from trn_agent_boot.trn_boot import _ntff_profile_via_ctypes

        mod.set_axon_ntff_profile_hook(
            _ntff_profile_via_ctypes("/opt/axon/libaxon_pjrt.so"))
    from concourse import bass_utils as bu

    bu.upload_artifacts = lambda tmpdir: str(tmpdir)


def kernel(x, w_attn, b_attn, w_proj, b_proj, _trace=False):
    x = np.asarray(x)
    w_attn = np.asarray(w_attn)
    b_attn = np.asarray(b_attn)
    w_proj = np.asarray(w_proj)
    b_proj = np.asarray(b_proj)

    if "nc" not in _cache:
        _cache["nc"] = _build()
    nc = _cache["nc"]

    scale = 1.0 / np.sqrt(HD)
    f16 = ml_dtypes.float16 if not hasattr(np, "float16") else np.float16
    tri = np.where(np.arange(128)[:, None] <= np.arange(128)[None, :],
                   np.float32(0.0), np.float32(NEG)).astype(np.float32)

    in_maps = []
    for core in range(8):
        b, hg = core // 2, core % 2
        qs = slice(hg * DL, (hg + 1) * DL)
        ks = slice(D + hg * DL, D + (hg + 1) * DL)
        vs = slice(2 * D + hg * DL, 2 * D + (hg + 1) * DL)
        wq = (w_attn[:, qs] * scale).astype(f16)
        wk = w_attn[:, ks].astype(f16)
        wqk_host = np.concatenate([wq, wk], axis=1)
        bqk_host = np.concatenate(
            [b_attn[qs] * scale, b_attn[ks]]).astype(np.float32)
        in_maps.append({
            "xT": np.ascontiguousarray(x[b].T).astype(f16),
            "wqk": np.ascontiguousarray(wqk_host),
            "bqk": np.ascontiguousarray(bqk_host.reshape(8, 128).T),
            "wv": np.ascontiguousarray(w_attn[:, vs]).astype(f16),
            "bv": np.ascontiguousarray(b_attn[vs][None, :]).astype(np.float32),
            "wp": np.ascontiguousarray(w_proj[hg * DL:(hg + 1) * DL, :]).astype(f16),
            "tri": tri,
        })

    kwargs = {}
    if _trace:
        _enable_trace_hooks()
        kwargs = dict(trace=True, trace_cores=[0])
    res = run_bass_kernel_spmd(nc, in_maps, core_ids=list(range(8)), **kwargs)

    outp = np.empty((B, T, D), np.float32)
    for b in range(B):
        outp[b] = res.results[2 * b]["out"] + res.results[2 * b + 1]["out"]
    outp += b_proj.astype(np.float32)

    if _trace:
        print(f"HW exec time: {res.exec_time_ns} ns")
    return outp

